# revision 1
# baseline (speedup 1.0000x reference)
"""Causal dilated 1D conv (KW=4, dilation=8) as shifted matmuls on 8 TRN2 cores.

out[b,o,t] = sum_{k,c} W[o, c*4+k] * x[b, c, t + k*8 - 24]

Sharding: data-parallel over batch (16 batches -> 2 per core). Each core runs
an identical program: all weights stationary in SBUF, x streamed in 512-wide
time blocks (+24 halo), 16 accumulating matmuls (4 c-chunks x 4 taps) per
(out-chunk, time-block) PSUM group, PSUM copied back via DVE and DMA'd out.

Matmuls run in float32r (fp32 data, FP22 multiply, fp32 accumulate): 1
cycle/row for free-dim >= 256, i.e. full bf16-class PE throughput on fp32
data, ~1.5e-4 relative error over the K=2048 contraction.

Startup: the first time-block's x tiles (1.1MB) are DMA'd before the 16
weight tiles (4MB, issued in first-group consumption order), so the PE
starts matmuls ~12us in and the HAM clock warms immediately. Steady state
measures ~232ns per matmul (streaming floor 213ns; the remainder is the
per-matmul fp32 weight reload, which walrus emits per MATMUL and cannot be
amortized), ~95% PE busy, ~500us total vs a ~437us PE-streaming floor.
"""

import numpy as np

B = 16
C_IN = 512
C_OUT = 512
T = 8192
KW = 4
DIL = 8
PAD = (KW - 1) * DIL  # 24

N_CORES = 8
B_PER = B // N_CORES  # 2
P = 128
TBLK = 512
NT = T // TBLK        # 16
NCC = C_IN // P       # 4
NOC = C_OUT // P      # 4

_cache = {}


def _build():
    import concourse.tile as tile
    from concourse import bacc, mybir

    nc = bacc.Bacc("TRN2", target_bir_lowering=False, debug=False,
                   num_devices=N_CORES)
    x = nc.dram_tensor("x", [B_PER, C_IN, T + PAD], mybir.dt.float32r,
                       kind="ExternalInput").ap()
    # weights pre-arranged on host as [cc, tap, c=128, o=512]
    wt = nc.dram_tensor("wt", [NCC, KW, P, C_OUT], mybir.dt.float32r,
                        kind="ExternalInput").ap()
    out = nc.dram_tensor("out", [B_PER, C_OUT, T], mybir.dt.float32,
                         kind="ExternalOutput").ap()
    f32 = mybir.dt.float32
    f32r = mybir.dt.float32r

    with tile.TileContext(nc) as tc:
        with tc.tile_pool(name="wpool", bufs=1) as wpool, \
             tc.tile_pool(name="xpool", bufs=8) as xpool, \
             tc.tile_pool(name="opool", bufs=8) as opool, \
             tc.tile_pool(name="pspool", bufs=8, space="PSUM") as pspool:

            def load_xt(b, tb):
                xts = []
                for cc in range(NCC):
                    xt = xpool.tile([P, TBLK + PAD], f32r,
                                    name=f"xt{cc}", tag=f"xt{cc}")
                    nc.sync.dma_start(
                        xt[:],
                        x[b, cc * P:(cc + 1) * P,
                          tb * TBLK: tb * TBLK + TBLK + PAD])
                    xts.append(xt)
                return xts

            # Interleave the first time-block's x tiles with their matching
            # per-cc weight tiles: the bootstrap fan-out consumes (cc=0,
            # k=0..3) first, so its inputs (xt0 + 4 weight tiles, ~1.3MB)
            # lead the wire stream and the PE starts ~10.5us in, fed
            # continuously while the remaining weights arrive.
            first_xts = []
            wtiles = [[None] * KW for _ in range(NCC)]
            for cc in range(NCC):
                xt = xpool.tile([P, TBLK + PAD], f32r,
                                name=f"xt{cc}", tag=f"xt{cc}")
                nc.sync.dma_start(xt[:], x[0, cc * P:(cc + 1) * P,
                                           0:TBLK + PAD])
                first_xts.append(xt)
                for k in range(KW):
                    wtile = wpool.tile([P, C_OUT], f32r, name=f"w_{cc}_{k}",
                                       tag=f"w_{cc}_{k}")
                    nc.sync.dma_start(wtile[:], wt[cc, k])
                    wtiles[cc][k] = wtile

            n_acc = NCC * KW
            cks = [(cc, k) for cc in range(NCC) for k in range(KW)]

            # Bootstrap block: emit MMs in weight-DMA-arrival order, fanning
            # each arriving weight across the 4 oc PSUM banks, so the in-order
            # PE stream is never head-of-line blocked on a later weight tile.
            pss0 = [pspool.tile([P, TBLK], f32, name="ps", tag="ps")
                    for _ in range(NOC)]
            for ci, (cc, k) in enumerate(cks):
                for oc in range(NOC):
                    nc.tensor.matmul(
                        pss0[oc][:],
                        wtiles[cc][k][:, oc * P:(oc + 1) * P],
                        first_xts[cc][:, k * DIL: k * DIL + TBLK],
                        start=(ci == 0),
                        stop=(ci == n_acc - 1),
                    )
            for oc in range(NOC):
                ot = opool.tile([P, TBLK], f32, name="ot", tag="ot")
                nc.vector.tensor_copy(ot[:], pss0[oc][:])
                nc.sync.dma_start(out[0, oc * P:(oc + 1) * P, 0:TBLK], ot[:])

            for b in range(B_PER):
                for tb in range(NT):
                    if b == 0 and tb == 0:
                        continue
                    xts = load_xt(b, tb)
                    for oc in range(NOC):
                        ps = pspool.tile([P, TBLK], f32, name="ps", tag="ps")
                        for ci, (cc, k) in enumerate(cks):
                            nc.tensor.matmul(
                                ps[:],
                                wtiles[cc][k][:, oc * P:(oc + 1) * P],
                                xts[cc][:, k * DIL: k * DIL + TBLK],
                                start=(ci == 0),
                                stop=(ci == n_acc - 1),
                            )
                        ot = opool.tile([P, TBLK], f32, name="ot", tag="ot")
                        nc.vector.tensor_copy(ot[:], ps[:])
                        nc.sync.dma_start(
                            out[b, oc * P:(oc + 1) * P,
                                tb * TBLK:(tb + 1) * TBLK],
                            ot[:])

    nc.compile()
    return nc


def _get_nc():
    if "nc" not in _cache:
        _cache["nc"] = _build()
    return _cache["nc"]


def _make_in_maps(x, W):
    xpad = np.pad(np.ascontiguousarray(x, dtype=np.float32),
                  ((0, 0), (0, 0), (PAD, 0)))
    w = np.ascontiguousarray(W, dtype=np.float32).reshape(C_OUT, C_IN, KW)
    # wt[cc, k, c, o] = W[o, (cc*128+c)*KW + k]
    wt = np.transpose(w.reshape(C_OUT, NCC, P, KW), (1, 3, 2, 0)).copy()
    return [{"x": np.ascontiguousarray(xpad[i * B_PER:(i + 1) * B_PER]),
             "wt": wt} for i in range(N_CORES)]


def kernel(x, W):
    from concourse.bass_utils import run_bass_kernel_spmd

    nc = _get_nc()
    in_maps = _make_in_maps(x, W)
    res = run_bass_kernel_spmd(nc, in_maps, list(range(N_CORES)))
    return np.concatenate([r["out"] for r in res.results], axis=0)



# revision 5
# speedup vs baseline: 1.0747x; 1.0747x over previous
"""Causal dilated 1D conv (KW=4, dilation=8) as shifted matmuls on 8 TRN2 cores.

out[b,o,t] = sum_{k,c} W[o, c*4+k] * x[b, c, t + k*8 - 24]

Sharding: data-parallel over batch (16 batches -> 2 per core). Each core runs
an identical program: all weights stationary in SBUF, x streamed in 512-wide
time blocks (+24 halo), 16 accumulating matmuls (4 c-chunks x 4 taps) per
(out-chunk, time-block) PSUM group, PSUM copied back via DVE and DMA'd out.

Matmuls run in bfloat16 (fp32 PSUM accumulate): 1 cycle/row streaming,
and unlike fp32/f32r the compiler-automatic Fast Weight Load path (FWL,
4 XBUSes, background weight buffer) is enabled, so the per-matmul
LDWEIGHTS hides under the previous matmul's 512-row stream. bf16
quantization of x and W gives ~2.3e-3 relative error over the K=2048
contraction (gate is 2e-2; fp8 DoubleRow measured 4e-2 -> unusable).

Startup: the first time-block's x tiles are DMA'd interleaved with the 16
weight tiles (2MB, issued in first-group consumption order), so the PE
starts matmuls early and the HAM clock warms immediately. Streaming floor
is 213ns per matmul (512 rows @ 2.4GHz), ~437us total.
"""

import ml_dtypes
import numpy as np

B = 16
C_IN = 512
C_OUT = 512
T = 8192
KW = 4
DIL = 8
PAD = (KW - 1) * DIL  # 24

N_CORES = 8
B_PER = B // N_CORES  # 2
P = 128
TBLK = 512
NT = T // TBLK        # 16
NCC = C_IN // P       # 4
NOC = C_OUT // P      # 4

_cache = {}


def _build():
    import concourse.tile as tile
    from concourse import bacc, mybir

    nc = bacc.Bacc("TRN2", target_bir_lowering=False, debug=False,
                   num_devices=N_CORES)
    x = nc.dram_tensor("x", [B_PER, C_IN, T + PAD], mybir.dt.bfloat16,
                       kind="ExternalInput").ap()
    # weights pre-arranged on host as [cc, tap, c=128, o=512]
    wt = nc.dram_tensor("wt", [NCC, KW, P, C_OUT], mybir.dt.bfloat16,
                        kind="ExternalInput").ap()
    out = nc.dram_tensor("out", [B_PER, C_OUT, T], mybir.dt.float32,
                         kind="ExternalOutput").ap()
    f32 = mybir.dt.float32
    bf16 = mybir.dt.bfloat16

    with tile.TileContext(nc) as tc:
        with tc.tile_pool(name="wpool", bufs=1) as wpool, \
             tc.tile_pool(name="xpool", bufs=8) as xpool, \
             tc.tile_pool(name="opool", bufs=8) as opool, \
             tc.tile_pool(name="pspool", bufs=8, space="PSUM") as pspool:

            def load_xt(b, tb):
                xts = []
                for cc in range(NCC):
                    xt = xpool.tile([P, TBLK + PAD], bf16,
                                    name=f"xt{cc}", tag=f"xt{cc}")
                    nc.sync.dma_start(
                        xt[:],
                        x[b, cc * P:(cc + 1) * P,
                          tb * TBLK: tb * TBLK + TBLK + PAD])
                    xts.append(xt)
                return xts

            # Interleave the first time-block's x tiles with their matching
            # per-cc weight tiles: the bootstrap fan-out consumes (cc=0,
            # k=0..3) first, so its inputs (xt0 + 4 weight tiles, ~1.3MB)
            # lead the wire stream and the PE starts ~10.5us in, fed
            # continuously while the remaining weights arrive.
            first_xts = []
            wtiles = [[None] * KW for _ in range(NCC)]
            for cc in range(NCC):
                xt = xpool.tile([P, TBLK + PAD], bf16,
                                name=f"xt{cc}", tag=f"xt{cc}")
                nc.sync.dma_start(xt[:], x[0, cc * P:(cc + 1) * P,
                                           0:TBLK + PAD])
                first_xts.append(xt)
                for k in range(KW):
                    wtile = wpool.tile([P, C_OUT], bf16, name=f"w_{cc}_{k}",
                                       tag=f"w_{cc}_{k}")
                    nc.sync.dma_start(wtile[:], wt[cc, k])
                    wtiles[cc][k] = wtile

            n_acc = NCC * KW
            cks = [(cc, k) for cc in range(NCC) for k in range(KW)]

            # Bootstrap block: emit MMs in weight-DMA-arrival order, fanning
            # each arriving weight across the 4 oc PSUM banks, so the in-order
            # PE stream is never head-of-line blocked on a later weight tile.
            pss0 = [pspool.tile([P, TBLK], f32, name="ps", tag="ps")
                    for _ in range(NOC)]
            for ci, (cc, k) in enumerate(cks):
                for oc in range(NOC):
                    nc.tensor.matmul(
                        pss0[oc][:],
                        wtiles[cc][k][:, oc * P:(oc + 1) * P],
                        first_xts[cc][:, k * DIL: k * DIL + TBLK],
                        start=(ci == 0),
                        stop=(ci == n_acc - 1),
                    )
            for oc in range(NOC):
                ot = opool.tile([P, TBLK], f32, name="ot", tag="ot")
                nc.vector.tensor_copy(ot[:], pss0[oc][:])
                nc.sync.dma_start(out[0, oc * P:(oc + 1) * P, 0:TBLK], ot[:])

            for b in range(B_PER):
                for tb in range(NT):
                    if b == 0 and tb == 0:
                        continue
                    xts = load_xt(b, tb)
                    for oc in range(NOC):
                        ps = pspool.tile([P, TBLK], f32, name="ps", tag="ps")
                        for ci, (cc, k) in enumerate(cks):
                            nc.tensor.matmul(
                                ps[:],
                                wtiles[cc][k][:, oc * P:(oc + 1) * P],
                                xts[cc][:, k * DIL: k * DIL + TBLK],
                                start=(ci == 0),
                                stop=(ci == n_acc - 1),
                            )
                        ot = opool.tile([P, TBLK], f32, name="ot", tag="ot")
                        nc.vector.tensor_copy(ot[:], ps[:])
                        nc.sync.dma_start(
                            out[b, oc * P:(oc + 1) * P,
                                tb * TBLK:(tb + 1) * TBLK],
                            ot[:])

    nc.compile()
    return nc


def _get_nc():
    if "nc" not in _cache:
        _cache["nc"] = _build()
    return _cache["nc"]


def _make_in_maps(x, W):
    xb = np.ascontiguousarray(x, dtype=np.float32).astype(ml_dtypes.bfloat16)
    xpad = np.pad(xb, ((0, 0), (0, 0), (PAD, 0)))
    w = np.ascontiguousarray(W, dtype=np.float32).reshape(C_OUT, C_IN, KW)
    # wt[cc, k, c, o] = W[o, (cc*128+c)*KW + k]
    wt = np.transpose(w.reshape(C_OUT, NCC, P, KW),
                      (1, 3, 2, 0)).astype(ml_dtypes.bfloat16).copy()
    return [{"x": np.ascontiguousarray(xpad[i * B_PER:(i + 1) * B_PER]),
             "wt": wt} for i in range(N_CORES)]


def kernel(x, W):
    from concourse.bass_utils import run_bass_kernel_spmd

    nc = _get_nc()
    in_maps = _make_in_maps(x, W)
    res = run_bass_kernel_spmd(nc, in_maps, list(range(N_CORES)))
    return np.concatenate([r["out"] for r in res.results], axis=0)

